# revision 33
# baseline (speedup 1.0000x reference)
"""Trainium2 Bass kernel for nn_DCM (dynamic conv module), data-parallel over
batch N=8 across 8 NeuronCores (1 sample per core).

Per-core program (sample n):
  x [512, 3600] bf16 (host-cast) in chunk-major layout
  for k in (1,3,5):
    f_k = relu(w1k' @ x + b1k)          (1x1 conv, BN scale folded into w)
    pooled_k = block-sums of x          (chunkwise 4x4-block DVE reductions,
                                         1/area folded into w2)
    g_k = relu(w2k'' @ pooled_k + b2k)  (tiny matmul)
    o_k = relu(depthwise(f_k, g_k))     (k^2 diag(g) matmuls on shifted
                                         zero-padded windows, PSUM accum;
                                         k=1 is a fused scale+relu on ACT)
    d_k = relu(wfk' @ o_k + bfk)
  y = relu(w_out' @ [x;d1;d3;d5] + b_out)  (16 K-tiles accumulated in PSUM)

All matmuls bf16 (fp32 PSUM accumulate). Weights are pre-transposed into
partition-major SBUF layouts, BN-folded and bf16-cast on the host so every
weight DMA is one contiguous descriptor per partition.
"""

import json

import numpy as np
import ml_dtypes

import concourse.bass as bass
import concourse.tile as tile
from concourse import mybir
from concourse.vector_clock import ScopedClock

P = 128
C = 512
C4 = 128
H = W = 60
HW = H * W
NB = 10          # bands
BR = 6           # rows per band
NT = BR * W      # 360 columns per band
CHUNK = 2 * NT   # x DMA chunk = 2 bands
NCHUNK = HW // CHUNK
CROWS = CHUNK // W  # rows per chunk (12)
N_CORES = 8
F32 = mybir.dt.float32
BF16 = mybir.dt.bfloat16
RELU = mybir.ActivationFunctionType.Relu

# ---------------------------------------------------------------------------
# Patches for walrus/concourse skew in this container: this walrus build only
# encodes ONE sync wait per instruction, while Tile emits several.
# 1) TileContext tail drain: emit its waits as 1-wait NOPs on SP instead.
# 2) to_json_bytes post-pass: split any instruction with N>1 waits into N-1
#    preceding same-engine 1-wait NOPs (same-engine program order makes this
#    semantically identical).
# ---------------------------------------------------------------------------


def _patched_drain_and_barrier(self, tick_clock, wait_clock):
    nc = self.nc
    probe = nc.sync.nop(nofuse=True)
    wait_clock.add_sem_waits(probe.ins, ScopedClock({None: tick_clock.global_clock}))
    si = probe.ins.sync_info
    waits = list(si.on_wait) if si is not None else []
    probe.ins.sync_info = mybir.SyncInfo(on_wait=[], on_update=list(si.on_update))

    # distribute the global-clock waits engine-affine (1-wait NOPs), then the
    # all-engine barrier transitively covers everything
    def eng_for(w):
        name = getattr(w, "ant_name", None) or ""
        if name.startswith("Activation"):
            return nc.scalar
        if name.startswith("DVE"):
            return nc.vector
        if name.startswith("PE"):
            return nc.tensor
        if name.startswith("Pool") or name.startswith("DMASW"):
            return nc.gpsimd
        return nc.sync

    for w in waits:
        n = eng_for(w).nop(nofuse=True)
        n.ins.sync_info = mybir.SyncInfo(on_wait=[w], on_update=[])
    nc.sync.drain()
    nc.all_engine_barrier()
    assert self.sems is not None
    popped = nc._tile_sem_poison_stack.pop()
    assert popped is self._sem_poison
    # Skip emitting the tail sem-clear/dma-reset instructions + second barrier
    # (~7us): the program preamble re-initializes semaphores on each
    # execution. Keep the allocator bookkeeping that clear_and_free did.
    sems = list(self.sems.allocated().values())
    sem_nums = [s.num if hasattr(s, "num") else s for s in sems]
    if sem_nums:
        nc._state.prepend_free_semaphores(sem_nums)
        for poison_set in nc._tile_sem_poison_stack:
            poison_set.update(sem_nums)


def _split_waits_json(raw: bytes) -> bytes:
    m = json.loads(raw)
    ctr = 0
    changed = False
    for f in m.get("functions", []):
        for bb in f.get("blocks", []):
            out = []
            for inst in bb.get("instructions", []):
                si = inst.get("sync_info")
                waits = (si or {}).get("on_wait") or []
                if len(waits) > 1:
                    changed = True
                    for w in waits[:-1]:
                        ctr += 1
                        nop = {
                            "engine": inst.get("engine"),
                            "ins": [],
                            "outs": [],
                            "name": f"{inst['name']}-sw{ctr}",
                            "opcode": "NoOp",
                            "sync_info": {"on_update": [], "on_wait": [w]},
                        }
                        if "debug" in inst:
                            nop["debug"] = inst["debug"]
                        out.append(nop)
                    si["on_wait"] = [waits[-1]]
                out.append(inst)
            bb["instructions"] = out
    return json.dumps(m).encode() if changed else raw


_PATCHED = False


def _apply_patches():
    global _PATCHED
    if _PATCHED:
        return
    tile.TileContext._drain_and_barrier = _patched_drain_and_barrier
    orig = bass.Bass.to_json_bytes

    def _patched_to_json_bytes(self, *a, **kw):
        return _split_waits_json(orig(self, *a, **kw))

    bass.Bass.to_json_bytes = _patched_to_json_bytes
    _PATCHED = True


# ---------------------------------------------------------------------------
# Bass program
# ---------------------------------------------------------------------------


def _build_bass():
    _apply_patches()
    nc = bass.Bass(trn_type="TRN2")

    # all inputs pre-arranged on host into partition-major layouts
    x_d = nc.dram_tensor("x", [NCHUNK, P, 4, CHUNK], BF16, kind="ExternalInput")
    w1_d = nc.dram_tensor("w1sb", [P, 3, 4, C4], BF16, kind="ExternalInput")
    w2_d = nc.dram_tensor("w2sb", [P, 3, 4, C4], BF16, kind="ExternalInput")
    wf_d = nc.dram_tensor("wfsb", [P, 3, C], BF16, kind="ExternalInput")
    wo_d = nc.dram_tensor("wosb", [P, 16, C], BF16, kind="ExternalInput")
    b1_d = nc.dram_tensor("b1sb", [P, 3], F32, kind="ExternalInput")
    b2_d = nc.dram_tensor("b2sb", [P, 3], F32, kind="ExternalInput")
    bf_d = nc.dram_tensor("bfsb", [P, 3, 4], F32, kind="ExternalInput")
    bo_d = nc.dram_tensor("bosb", [P, 4], F32, kind="ExternalInput")
    id_d = nc.dram_tensor("ident", [P, P], F32, kind="ExternalInput")
    y_d = nc.dram_tensor("y", [C, HW], F32, kind="ExternalOutput")

    with tile.TileContext(nc) as tc:
        with (
            tc.tile_pool(name="consts", bufs=1) as consts,
            tc.tile_pool(name="xpool", bufs=1) as xpool,
            tc.tile_pool(name="fpool", bufs=1) as fpool,
            tc.tile_pool(name="ptmp", bufs=2) as ptmp,
            tc.tile_pool(name="gpool", bufs=1) as gpool,
            tc.tile_pool(name="obuf", bufs=3) as obuf,
            tc.tile_pool(name="dbuf", bufs=3) as dbuf,
            tc.tile_pool(name="ybuf", bufs=3) as ybuf,
            tc.tile_pool(name="psum", bufs=4, space="PSUM") as psum,
        ):
            # ---- weights / constants -> SBUF ----
            # w1 + x chunks on the Sync HWDGE queue (critical path, in order);
            # small biases first on gpsimd, bulkier weights after
            w1T = consts.tile([P, 3, 4, C4], BF16)
            nc.sync.dma_start(w1T[:], w1_d[:])
            b1 = consts.tile([P, 3], F32)
            nc.gpsimd.dma_start(b1[:], b1_d[:])
            b2 = consts.tile([P, 3], F32)
            nc.gpsimd.dma_start(b2[:], b2_d[:])
            bfb = consts.tile([P, 3, 4], F32)
            nc.gpsimd.dma_start(bfb[:], bf_d[:])
            bo = consts.tile([P, 4], F32)
            nc.gpsimd.dma_start(bo[:], bo_d[:])
            ident = consts.tile([P, P], F32)
            nc.gpsimd.dma_start(ident[:], id_d[:])
            w2T = consts.tile([P, 3, 4, C4], BF16)
            nc.gpsimd.dma_start(w2T[:], w2_d[:])
            wfT = consts.tile([P, 3, C], BF16)
            nc.gpsimd.dma_start(wfT[:], wf_d[:])

            # ---- x -> SBUF (chunk-major, contiguous per partition) ----
            # split across the two HWDGE issuing engines (sync + scalar): each
            # engine's transfers land on its own HW queue (~195 GB/s apiece).
            # The first DMA is just band 0 so the f-stage starts early.
            x_sb = xpool.tile([P, NCHUNK, 4, CHUNK], BF16)
            nc.scalar.dma_start(x_sb[:, 0, :, 0:NT], x_d[0][:, :, 0:NT])
            nc.scalar.dma_start(x_sb[:, 0, :, NT:], x_d[0][:, :, NT:])
            nc.sync.dma_start(x_sb[:, 1], x_d[1])
            nc.scalar.dma_start(x_sb[:, 2], x_d[2])
            nc.sync.dma_start(x_sb[:, 3], x_d[3])
            nc.scalar.dma_start(x_sb[:, 4], x_d[4])
            # final-conv weights not needed until the band loop; load after x
            woT = consts.tile([P, 16, C], BF16)
            nc.sync.dma_start(woT[:], wo_d[:])

            def xsl(kt, b):
                """x band slice [P, NT] for band b, K-tile kt."""
                return x_sb[:, b // 2, kt, (b % 2) * NT:(b % 2) * NT + NT]

            # ---- f convs (k=1 plain, k=3/5 zero-padded layouts) ----
            # band-outer so each arriving x chunk feeds 3 convs' worth of PE
            f1 = fpool.tile([P, HW], BF16)
            f3 = fpool.tile([P, 64, 64], BF16)
            f5 = fpool.tile([P, 64, 64], BF16)
            for fpad in (f3, f5):  # zero only the halo border strips
                nc.vector.memset(fpad[:, 0:2, :], 0.0)
                nc.vector.memset(fpad[:, 62:64, :], 0.0)
                nc.vector.memset(fpad[:, 2:62, 0:2], 0.0)
                nc.vector.memset(fpad[:, 2:62, 62:64], 0.0)
            def emit_f_band(b):
                for ki, fdst in ((0, f1), (1, f3), (2, f5)):
                    ps = psum.tile([P, NT], F32, tag="work", name=f"fps{b}{ki}")
                    for kt in range(4):
                        nc.tensor.matmul(ps[:], w1T[:, ki, kt, :], xsl(kt, b),
                                         start=(kt == 0), stop=(kt == 3))
                    if ki == 0:
                        dst = fdst[:, b * NT:(b + 1) * NT]
                    else:
                        dst = fdst[:, 2 + b * BR: 2 + (b + 1) * BR, 2:62]
                    nc.scalar.activation(dst, ps[:], RELU,
                                         bias=b1[:, ki:ki + 1], scale=1.0)

            for b in range(NB - 2):
                emit_f_band(b)

            # ---- pooling: direct 4x4 block sums, chunkwise (DVE) ----
            pooled = {k: gpool.tile([P, 4, k * k], BF16, name=f"pooled{k}")
                      for k in (1, 3, 5)}
            qs = [ptmp.tile([P, 15, 15], F32, name=f"q_{kt}", tag=f"q_{kt}")
                  for kt in range(4)]  # [wb][hb]
            for cb in range(NCHUNK):
                for kt in range(4):
                    nc.vector.reduce_sum(
                        qs[kt][:, :, cb * 3:(cb + 1) * 3],
                        x_sb[:, cb, kt, :].rearrange(
                            "p (hbl h wb w) -> p wb hbl h w",
                            hbl=3, h=4, wb=15, w=4),
                        axis=mybir.AxisListType.XY)

            g_sb = {}
            diag = {}

            def emit_pool(k):
                # one fused XY reduce per kt: q [wb][hb] -> pooled[k] [i][j]
                with nc.allow_low_precision(reason="pooled block sums in bf16"):
                    for kt in range(4):
                        nc.vector.reduce_sum(
                            pooled[k][:, kt, :].rearrange(
                                "p (i j) -> p i j", i=k),
                            qs[kt].rearrange(
                                "p (wbB wb) (hbB hb) -> p hbB wbB wb hb",
                                wbB=k, hbB=k),
                            axis=mybir.AxisListType.XY)

            def emit_g(k, ki, diag_engine=None):
                gp = psum.tile([P, k * k], F32, tag="work", name=f"gp{k}")
                for kt in range(4):
                    nc.tensor.matmul(gp[:], w2T[:, ki, kt, :], pooled[k][:, kt, :],
                                     start=(kt == 0), stop=(kt == 3))
                g = gpool.tile([P, k * k], F32, name=f"g{k}")
                nc.scalar.activation(g[:], gp[:], RELU,
                                     bias=b2[:, ki:ki + 1], scale=1.0)
                g_sb[k] = g
                if diag_engine is not None:
                    # diag tiles via broadcast multiply, in <=13-tap pieces so
                    # the first taps can start before the whole set is built:
                    # dg[p, t, c] = ident[p, c] * g[p, t]
                    slices = []
                    t0 = 0
                    while t0 < k * k:
                        n = min(13, k * k - t0)
                        dg = gpool.tile([P, n, P], BF16, name=f"diag{k}_{t0}")
                        diag_engine.tensor_tensor(
                            dg[:],
                            ident[:, None, :].to_broadcast((P, n, P)),
                            g[:, t0:t0 + n, None].to_broadcast((P, n, P)),
                            mybir.AluOpType.mult)
                        slices.extend(dg[:, i, :] for i in range(n))
                        t0 += n
                    diag[k] = slices

            # k=3 path first (earliest consumer: band-0 taps), then 5, then 1;
            # the last two f bands are interleaved between the g stages so the
            # PE has work while the g -> diag chains run on ACT/DVE
            emit_pool(3)
            emit_g(3, 1, nc.vector)
            emit_f_band(NB - 2)
            emit_pool(5)
            emit_g(5, 2, nc.vector)
            emit_f_band(NB - 1)
            with nc.allow_low_precision(reason="pooled block sums in bf16"):
                for kt in range(4):
                    nc.vector.reduce_sum(
                        pooled[1][:, kt, :],
                        qs[kt].rearrange("p a b -> p (a b)"),
                        axis=mybir.AxisListType.X)
            emit_g(1, 0)

            # ---- band loop ----
            for b in range(NB):
                # depthwise taps (k=3, k=5) accumulate in PSUM
                o_sb = {}
                for k, fpad in ((3, f3), (5, f5)):
                    pad = (k - 1) // 2
                    ps = psum.tile([P, NT], F32, tag="work")
                    t = 0
                    for i in range(k):
                        for j in range(k):
                            r0 = 2 + b * BR + i - pad
                            c0 = 2 + j - pad
                            nc.tensor.matmul(
                                ps[:], diag[k][t],
                                fpad[:, r0:r0 + BR, c0:c0 + W],
                                start=(t == 0), stop=(t == k * k - 1))
                            t += 1
                    o = obuf.tile([P, NT], BF16, tag=f"o{k}")
                    nc.scalar.activation(o[:], ps[:], RELU, bias=0.0, scale=1.0)
                    o_sb[k] = o
                # k=1: o1 = relu(g1 * f1)
                o1 = obuf.tile([P, NT], BF16, tag="o1")
                nc.scalar.activation(o1[:], f1[:, b * NT:(b + 1) * NT], RELU,
                                     bias=0.0, scale=g_sb[1][:, 0:1])
                o_sb[1] = o1

                # final-conv accumulators; the x-part MMs are interleaved with
                # the d-conv MMs so PSUM "work" slots recycle without stalling
                # the PE on the d epilogues (DVE/ACT alternating)
                op = [psum.tile([P, NT], F32, tag="out", name=f"op{b}_{m}")
                      for m in range(4)]
                d_tiles = {}

                def emit_x_mm(i):
                    kt, m = divmod(i, 4)
                    nc.tensor.matmul(op[m], woT[:, kt, m * P:(m + 1) * P],
                                     xsl(kt, b), start=(kt == 0), stop=False)

                def emit_d_mm(j):
                    pi, m = divmod(j, 4)
                    k = (3, 5, 1)[pi]
                    if m == 0:
                        d_tiles[k] = dbuf.tile([P, 4, NT], BF16, tag="d",
                                               name=f"d{b}_{k}")
                    d_sb = d_tiles[k]
                    dps = psum.tile([P, NT], F32, tag="work", name=f"dps{b}{j}")
                    ki = {1: 0, 3: 1, 5: 2}[k]
                    nc.tensor.matmul(dps[:], wfT[:, ki, m * P:(m + 1) * P],
                                     o_sb[k][:], start=True, stop=True)
                    if j % 2 == 0:
                        nc.vector.tensor_scalar(
                            d_sb[:, m, :], dps[:],
                            bfb[:, ki, m:m + 1], 0.0,
                            op0=mybir.AluOpType.add, op1=mybir.AluOpType.max)
                    else:
                        nc.scalar.activation(
                            d_sb[:, m, :], dps[:], RELU,
                            bias=bfb[:, ki, m:m + 1], scale=1.0)

                xi = di = 0
                while xi < 16 or di < 12:
                    if xi < 16 and (di >= 12 or xi * 12 <= di * 16):
                        emit_x_mm(xi)
                        xi += 1
                    else:
                        emit_d_mm(di)
                        di += 1
                for pi, k in enumerate((1, 3, 5)):
                    d_sb = d_tiles[k]
                    for ktl in range(4):
                        kt = 4 * (pi + 1) + ktl
                        for m in range(4):
                            nc.tensor.matmul(
                                op[m], woT[:, kt, m * P:(m + 1) * P],
                                d_sb[:, ktl, :],
                                start=False, stop=(pi == 2 and ktl == 3))
                # epilogue + store (alternate engines)
                ysb = ybuf.tile([P, 4, NT], F32, tag="y")
                for m in range(4):
                    if m % 2 == 0:
                        nc.vector.tensor_scalar(
                            ysb[:, m, :], op[m], bo[:, m:m + 1], 0.0,
                            op0=mybir.AluOpType.add, op1=mybir.AluOpType.max)
                    else:
                        nc.scalar.activation(ysb[:, m, :], op[m], RELU,
                                             bias=bo[:, m:m + 1], scale=1.0)
                    dma_eng = nc.sync if m % 2 == 0 else nc.scalar
                    dma_eng.dma_start(
                        y_d[m * P:(m + 1) * P, b * NT:(b + 1) * NT],
                        ysb[:, m, :])
    return nc


# ---------------------------------------------------------------------------
# Host side
# ---------------------------------------------------------------------------

_NC_CACHE = {}


def _get_nc():
    if "nc" not in _NC_CACHE:
        _NC_CACHE["nc"] = _build_bass()
    return _NC_CACHE["nc"]


def _host_prep(inputs):
    """Fold BN scales into weights, transpose into partition-major SBUF
    layouts, cast bf16."""
    bf16 = ml_dtypes.bfloat16
    f32 = np.float32

    def A(name):
        return np.asarray(inputs[name], f32)

    # lhsT for conv1: [K=C, M=C4] per k -> [P, 3, 4, C4]
    w1T = np.stack([(A(f"s1_{k}")[:, None] * A(f"w1_{k}")).T for k in (1, 3, 5)])
    w1sb = w1T.reshape(3, 4, P, C4).transpose(2, 0, 1, 3)
    areas = {1: 3600.0, 3: 400.0, 5: 144.0}
    w2T = np.stack([((A(f"s2_{k}")[:, None] * A(f"w2_{k}")) / areas[k]).T
                    for k in (1, 3, 5)])
    w2sb = w2T.reshape(3, 4, P, C4).transpose(2, 0, 1, 3)
    # lhsT for d conv: [K=C4, M=C] per k -> [P, 3, C]
    wfT = np.stack([(A(f"sf_{k}")[:, None] * A(f"wf_{k}")).T for k in (1, 3, 5)])
    wfsb = wfT.transpose(1, 0, 2)
    # lhsT for out conv: [K=4C, M=C] -> [P, 16, C]
    woT = (A("s_out")[:, None] * A("w_out")).T
    wosb = woT.reshape(16, P, C).transpose(1, 0, 2)

    b1sb = np.stack([A(f"b1_{k}") for k in (1, 3, 5)]).T
    b2sb = np.stack([A(f"b2_{k}") for k in (1, 3, 5)]).T
    bfsb = np.stack([A(f"bf_{k}").reshape(4, P) for k in (1, 3, 5)]).transpose(2, 0, 1)
    bosb = A("b_out").reshape(4, P).T
    return {
        "w1sb": np.ascontiguousarray(w1sb).astype(bf16),
        "w2sb": np.ascontiguousarray(w2sb).astype(bf16),
        "wfsb": np.ascontiguousarray(wfsb).astype(bf16),
        "wosb": np.ascontiguousarray(wosb).astype(bf16),
        "b1sb": np.ascontiguousarray(b1sb),
        "b2sb": np.ascontiguousarray(b2sb),
        "bfsb": np.ascontiguousarray(bfsb),
        "bosb": np.ascontiguousarray(bosb),
        "ident": np.eye(P, dtype=f32),
    }


def _host_x(x):
    """[512, 3600] fp32 -> chunk-major [NCHUNK, P, 4, CHUNK] bf16."""
    xb = x.astype(ml_dtypes.bfloat16)
    # row = kt*128 + p ; col = cb*CHUNK + w
    return np.ascontiguousarray(
        xb.reshape(4, P, NCHUNK, CHUNK).transpose(2, 1, 0, 3))


def _run(inputs, **kwargs):
    from concourse.bass_utils import run_bass_kernel_spmd

    common = _host_prep(inputs)
    x = np.asarray(inputs["x"], np.float32).reshape(N_CORES, C, HW)
    in_maps = [{**common, "x": _host_x(x[n])} for n in range(N_CORES)]
    return run_bass_kernel_spmd(_get_nc(), in_maps,
                                core_ids=list(range(N_CORES)), **kwargs)


def kernel(**inputs):
    res = _run(inputs)
    return np.stack([r["y"].reshape(C, H, W) for r in res.results]).astype(np.float32)


# revision 34
# speedup vs baseline: 1.0417x; 1.0417x over previous
"""Trainium2 Bass kernel for nn_DCM (dynamic conv module), data-parallel over
batch N=8 across 8 NeuronCores (1 sample per core).

Per-core program (sample n):
  x [512, 3600] bf16 (host-cast) in chunk-major layout
  for k in (1,3,5):
    f_k = relu(w1k' @ x + b1k)          (1x1 conv, BN scale folded into w)
    pooled_k = block-sums of x          (chunkwise 4x4-block DVE reductions,
                                         1/area folded into w2)
    g_k = relu(w2k'' @ pooled_k + b2k)  (tiny matmul)
    o_k = relu(depthwise(f_k, g_k))     (k^2 diag(g) matmuls on shifted
                                         zero-padded windows, PSUM accum;
                                         k=1 is a fused scale+relu on ACT)
    d_k = relu(wfk' @ o_k + bfk)
  y = relu(w_out' @ [x;d1;d3;d5] + b_out)  (16 K-tiles accumulated in PSUM)

All matmuls bf16 (fp32 PSUM accumulate). Weights are pre-transposed into
partition-major SBUF layouts, BN-folded and bf16-cast on the host so every
weight DMA is one contiguous descriptor per partition.
"""

import json

import numpy as np
import ml_dtypes

import concourse.bass as bass
import concourse.tile as tile
from concourse import mybir
from concourse.vector_clock import ScopedClock

P = 128
C = 512
C4 = 128
H = W = 60
HW = H * W
NB = 10          # bands
BR = 6           # rows per band
NT = BR * W      # 360 columns per band
CHUNK = 2 * NT   # x DMA chunk = 2 bands
NCHUNK = HW // CHUNK
CROWS = CHUNK // W  # rows per chunk (12)
N_CORES = 8
F32 = mybir.dt.float32
BF16 = mybir.dt.bfloat16
RELU = mybir.ActivationFunctionType.Relu

# ---------------------------------------------------------------------------
# Patches for walrus/concourse skew in this container: this walrus build only
# encodes ONE sync wait per instruction, while Tile emits several.
# 1) TileContext tail drain: emit its waits as 1-wait NOPs on SP instead.
# 2) to_json_bytes post-pass: split any instruction with N>1 waits into N-1
#    preceding same-engine 1-wait NOPs (same-engine program order makes this
#    semantically identical).
# ---------------------------------------------------------------------------


def _patched_drain_and_barrier(self, tick_clock, wait_clock):
    nc = self.nc
    probe = nc.sync.nop(nofuse=True)
    wait_clock.add_sem_waits(probe.ins, ScopedClock({None: tick_clock.global_clock}))
    si = probe.ins.sync_info
    waits = list(si.on_wait) if si is not None else []
    probe.ins.sync_info = mybir.SyncInfo(on_wait=[], on_update=list(si.on_update))

    # distribute the global-clock waits engine-affine (1-wait NOPs), then the
    # all-engine barrier transitively covers everything
    def eng_for(w):
        name = getattr(w, "ant_name", None) or ""
        if name.startswith("Activation"):
            return nc.scalar
        if name.startswith("DVE"):
            return nc.vector
        if name.startswith("PE"):
            return nc.tensor
        if name.startswith("Pool") or name.startswith("DMASW"):
            return nc.gpsimd
        return nc.sync

    for w in waits:
        n = eng_for(w).nop(nofuse=True)
        n.ins.sync_info = mybir.SyncInfo(on_wait=[w], on_update=[])
    nc.sync.drain()
    nc.all_engine_barrier()
    assert self.sems is not None
    popped = nc._tile_sem_poison_stack.pop()
    assert popped is self._sem_poison
    # Skip emitting the tail sem-clear/dma-reset instructions + second barrier
    # (~7us): the program preamble re-initializes semaphores on each
    # execution. Keep the allocator bookkeeping that clear_and_free did.
    sems = list(self.sems.allocated().values())
    sem_nums = [s.num if hasattr(s, "num") else s for s in sems]
    if sem_nums:
        nc._state.prepend_free_semaphores(sem_nums)
        for poison_set in nc._tile_sem_poison_stack:
            poison_set.update(sem_nums)


def _split_waits_json(raw: bytes) -> bytes:
    m = json.loads(raw)
    ctr = 0
    changed = False
    for f in m.get("functions", []):
        for bb in f.get("blocks", []):
            out = []
            for inst in bb.get("instructions", []):
                si = inst.get("sync_info")
                waits = (si or {}).get("on_wait") or []
                if len(waits) > 1:
                    changed = True
                    for w in waits[:-1]:
                        ctr += 1
                        nop = {
                            "engine": inst.get("engine"),
                            "ins": [],
                            "outs": [],
                            "name": f"{inst['name']}-sw{ctr}",
                            "opcode": "NoOp",
                            "sync_info": {"on_update": [], "on_wait": [w]},
                        }
                        if "debug" in inst:
                            nop["debug"] = inst["debug"]
                        out.append(nop)
                    si["on_wait"] = [waits[-1]]
                out.append(inst)
            bb["instructions"] = out
    return json.dumps(m).encode() if changed else raw


_PATCHED = False


def _apply_patches():
    global _PATCHED
    if _PATCHED:
        return
    tile.TileContext._drain_and_barrier = _patched_drain_and_barrier
    orig = bass.Bass.to_json_bytes

    def _patched_to_json_bytes(self, *a, **kw):
        return _split_waits_json(orig(self, *a, **kw))

    bass.Bass.to_json_bytes = _patched_to_json_bytes
    _PATCHED = True


# ---------------------------------------------------------------------------
# Bass program
# ---------------------------------------------------------------------------


def _build_bass():
    _apply_patches()
    nc = bass.Bass(trn_type="TRN2")

    # all inputs pre-arranged on host into partition-major layouts
    x_d = nc.dram_tensor("x", [NCHUNK, P, 4, CHUNK], BF16, kind="ExternalInput")
    w1_d = nc.dram_tensor("w1sb", [P, 3, 4, C4], BF16, kind="ExternalInput")
    w2_d = nc.dram_tensor("w2sb", [P, 3, 4, C4], BF16, kind="ExternalInput")
    wf_d = nc.dram_tensor("wfsb", [P, 3, C], BF16, kind="ExternalInput")
    wo_d = nc.dram_tensor("wosb", [P, 16, C], BF16, kind="ExternalInput")
    b1_d = nc.dram_tensor("b1sb", [P, 3], F32, kind="ExternalInput")
    b2_d = nc.dram_tensor("b2sb", [P, 3], F32, kind="ExternalInput")
    bf_d = nc.dram_tensor("bfsb", [P, 3, 4], F32, kind="ExternalInput")
    bo_d = nc.dram_tensor("bosb", [P, 4], F32, kind="ExternalInput")
    id_d = nc.dram_tensor("ident", [P, P], F32, kind="ExternalInput")
    y_d = nc.dram_tensor("y", [C, HW], F32, kind="ExternalOutput")

    with tile.TileContext(nc) as tc:
        with (
            tc.tile_pool(name="consts", bufs=1) as consts,
            tc.tile_pool(name="xpool", bufs=1) as xpool,
            tc.tile_pool(name="fpool", bufs=1) as fpool,
            tc.tile_pool(name="ptmp", bufs=2) as ptmp,
            tc.tile_pool(name="gpool", bufs=1) as gpool,
            tc.tile_pool(name="obuf", bufs=3) as obuf,
            tc.tile_pool(name="dbuf", bufs=3) as dbuf,
            tc.tile_pool(name="ybuf", bufs=3) as ybuf,
            tc.tile_pool(name="psum", bufs=4, space="PSUM") as psum,
        ):
            # ---- weights / constants -> SBUF ----
            # w1 + x chunks on the Sync HWDGE queue (critical path, in order);
            # small biases first on gpsimd, bulkier weights after
            w1T = consts.tile([P, 3, 4, C4], BF16)
            nc.sync.dma_start(w1T[:], w1_d[:])
            b1 = consts.tile([P, 3], F32)
            nc.gpsimd.dma_start(b1[:], b1_d[:])
            b2 = consts.tile([P, 3], F32)
            nc.gpsimd.dma_start(b2[:], b2_d[:])
            bfb = consts.tile([P, 3, 4], F32)
            nc.gpsimd.dma_start(bfb[:], bf_d[:])
            bo = consts.tile([P, 4], F32)
            nc.gpsimd.dma_start(bo[:], bo_d[:])
            ident = consts.tile([P, P], F32)
            nc.gpsimd.dma_start(ident[:], id_d[:])
            w2T = consts.tile([P, 3, 4, C4], BF16)
            nc.gpsimd.dma_start(w2T[:], w2_d[:])
            wfT = consts.tile([P, 3, C], BF16)
            nc.gpsimd.dma_start(wfT[:], wf_d[:])

            # ---- x -> SBUF (chunk-major, contiguous per partition) ----
            # split across the two HWDGE issuing engines (sync + scalar): each
            # engine's transfers land on its own HW queue (~195 GB/s apiece).
            # The first DMA is just band 0 so the f-stage starts early.
            x_sb = xpool.tile([P, NCHUNK, 4, CHUNK], BF16)
            nc.sync.dma_start(x_sb[:, 0], x_d[0])
            nc.scalar.dma_start(x_sb[:, 1], x_d[1])
            nc.sync.dma_start(x_sb[:, 2], x_d[2])
            nc.scalar.dma_start(x_sb[:, 3], x_d[3])
            nc.sync.dma_start(x_sb[:, 4], x_d[4])
            # final-conv weights not needed until the band loop; load after x
            woT = consts.tile([P, 16, C], BF16)
            nc.sync.dma_start(woT[:], wo_d[:])

            def xsl(kt, b):
                """x band slice [P, NT] for band b, K-tile kt."""
                return x_sb[:, b // 2, kt, (b % 2) * NT:(b % 2) * NT + NT]

            # ---- f convs (k=1 plain, k=3/5 zero-padded layouts) ----
            # band-outer so each arriving x chunk feeds 3 convs' worth of PE
            f1 = fpool.tile([P, HW], BF16)
            f3 = fpool.tile([P, 64, 64], BF16)
            f5 = fpool.tile([P, 64, 64], BF16)
            for fpad in (f3, f5):  # zero only the halo border strips
                nc.vector.memset(fpad[:, 0:2, :], 0.0)
                nc.vector.memset(fpad[:, 62:64, :], 0.0)
                nc.vector.memset(fpad[:, 2:62, 0:2], 0.0)
                nc.vector.memset(fpad[:, 2:62, 62:64], 0.0)
            def emit_f_band(b):
                for ki, fdst in ((0, f1), (1, f3), (2, f5)):
                    ps = psum.tile([P, NT], F32, tag="work", name=f"fps{b}{ki}")
                    for kt in range(4):
                        nc.tensor.matmul(ps[:], w1T[:, ki, kt, :], xsl(kt, b),
                                         start=(kt == 0), stop=(kt == 3))
                    if ki == 0:
                        dst = fdst[:, b * NT:(b + 1) * NT]
                    else:
                        dst = fdst[:, 2 + b * BR: 2 + (b + 1) * BR, 2:62]
                    nc.scalar.activation(dst, ps[:], RELU,
                                         bias=b1[:, ki:ki + 1], scale=1.0)

            for b in range(NB - 2):
                emit_f_band(b)

            # ---- pooling: direct 4x4 block sums, chunkwise (DVE) ----
            pooled = {k: gpool.tile([P, 4, k * k], BF16, name=f"pooled{k}")
                      for k in (1, 3, 5)}
            qs = [ptmp.tile([P, 15, 15], F32, name=f"q_{kt}", tag=f"q_{kt}")
                  for kt in range(4)]  # [wb][hb]
            for cb in range(NCHUNK):
                for kt in range(4):
                    nc.vector.reduce_sum(
                        qs[kt][:, :, cb * 3:(cb + 1) * 3],
                        x_sb[:, cb, kt, :].rearrange(
                            "p (hbl h wb w) -> p wb hbl h w",
                            hbl=3, h=4, wb=15, w=4),
                        axis=mybir.AxisListType.XY)

            g_sb = {}
            diag = {}

            def emit_pool(k):
                # one fused XY reduce per kt: q [wb][hb] -> pooled[k] [i][j]
                with nc.allow_low_precision(reason="pooled block sums in bf16"):
                    for kt in range(4):
                        nc.vector.reduce_sum(
                            pooled[k][:, kt, :].rearrange(
                                "p (i j) -> p i j", i=k),
                            qs[kt].rearrange(
                                "p (wbB wb) (hbB hb) -> p hbB wbB wb hb",
                                wbB=k, hbB=k),
                            axis=mybir.AxisListType.XY)

            def emit_g(k, ki, diag_engine=None):
                gp = psum.tile([P, k * k], F32, tag="work", name=f"gp{k}")
                for kt in range(4):
                    nc.tensor.matmul(gp[:], w2T[:, ki, kt, :], pooled[k][:, kt, :],
                                     start=(kt == 0), stop=(kt == 3))
                g = gpool.tile([P, k * k], F32, name=f"g{k}")
                nc.scalar.activation(g[:], gp[:], RELU,
                                     bias=b2[:, ki:ki + 1], scale=1.0)
                g_sb[k] = g
                if diag_engine is not None:
                    # diag tiles via broadcast multiply, in <=13-tap pieces so
                    # the first taps can start before the whole set is built:
                    # dg[p, t, c] = ident[p, c] * g[p, t]
                    slices = []
                    t0 = 0
                    while t0 < k * k:
                        n = min(13, k * k - t0)
                        dg = gpool.tile([P, n, P], BF16, name=f"diag{k}_{t0}")
                        diag_engine.tensor_tensor(
                            dg[:],
                            ident[:, None, :].to_broadcast((P, n, P)),
                            g[:, t0:t0 + n, None].to_broadcast((P, n, P)),
                            mybir.AluOpType.mult)
                        slices.extend(dg[:, i, :] for i in range(n))
                        t0 += n
                    diag[k] = slices

            # k=3 path first (earliest consumer: band-0 taps), then 5, then 1;
            # the last two f bands are interleaved between the g stages so the
            # PE has work while the g -> diag chains run on ACT/DVE
            emit_pool(3)
            emit_g(3, 1, nc.vector)
            emit_f_band(NB - 2)
            emit_pool(5)
            emit_g(5, 2, nc.vector)
            emit_f_band(NB - 1)
            with nc.allow_low_precision(reason="pooled block sums in bf16"):
                for kt in range(4):
                    nc.vector.reduce_sum(
                        pooled[1][:, kt, :],
                        qs[kt].rearrange("p a b -> p (a b)"),
                        axis=mybir.AxisListType.X)
            emit_g(1, 0)

            # ---- band loop ----
            for b in range(NB):
                # depthwise taps (k=3, k=5) accumulate in PSUM
                o_sb = {}
                for k, fpad in ((3, f3), (5, f5)):
                    pad = (k - 1) // 2
                    ps = psum.tile([P, NT], F32, tag="work")
                    t = 0
                    for i in range(k):
                        for j in range(k):
                            r0 = 2 + b * BR + i - pad
                            c0 = 2 + j - pad
                            nc.tensor.matmul(
                                ps[:], diag[k][t],
                                fpad[:, r0:r0 + BR, c0:c0 + W],
                                start=(t == 0), stop=(t == k * k - 1))
                            t += 1
                    o = obuf.tile([P, NT], BF16, tag=f"o{k}")
                    nc.scalar.activation(o[:], ps[:], RELU, bias=0.0, scale=1.0)
                    o_sb[k] = o
                # k=1: o1 = relu(g1 * f1)
                o1 = obuf.tile([P, NT], BF16, tag="o1")
                nc.scalar.activation(o1[:], f1[:, b * NT:(b + 1) * NT], RELU,
                                     bias=0.0, scale=g_sb[1][:, 0:1])
                o_sb[1] = o1

                # final-conv accumulators; the x-part MMs are interleaved with
                # the d-conv MMs so PSUM "work" slots recycle without stalling
                # the PE on the d epilogues (DVE/ACT alternating)
                op = [psum.tile([P, NT], F32, tag="out", name=f"op{b}_{m}")
                      for m in range(4)]
                d_tiles = {}

                def emit_x_mm(i):
                    kt, m = divmod(i, 4)
                    nc.tensor.matmul(op[m], woT[:, kt, m * P:(m + 1) * P],
                                     xsl(kt, b), start=(kt == 0), stop=False)

                def emit_d_mm(j):
                    pi, m = divmod(j, 4)
                    k = (3, 5, 1)[pi]
                    if m == 0:
                        d_tiles[k] = dbuf.tile([P, 4, NT], BF16, tag="d",
                                               name=f"d{b}_{k}")
                    d_sb = d_tiles[k]
                    dps = psum.tile([P, NT], F32, tag="work", name=f"dps{b}{j}")
                    ki = {1: 0, 3: 1, 5: 2}[k]
                    nc.tensor.matmul(dps[:], wfT[:, ki, m * P:(m + 1) * P],
                                     o_sb[k][:], start=True, stop=True)
                    if j % 2 == 0:
                        nc.vector.tensor_scalar(
                            d_sb[:, m, :], dps[:],
                            bfb[:, ki, m:m + 1], 0.0,
                            op0=mybir.AluOpType.add, op1=mybir.AluOpType.max)
                    else:
                        nc.scalar.activation(
                            d_sb[:, m, :], dps[:], RELU,
                            bias=bfb[:, ki, m:m + 1], scale=1.0)

                xi = di = 0
                while xi < 16 or di < 12:
                    if xi < 16 and (di >= 12 or xi * 12 <= di * 16):
                        emit_x_mm(xi)
                        xi += 1
                    else:
                        emit_d_mm(di)
                        di += 1
                for pi, k in enumerate((1, 3, 5)):
                    d_sb = d_tiles[k]
                    for ktl in range(4):
                        kt = 4 * (pi + 1) + ktl
                        for m in range(4):
                            nc.tensor.matmul(
                                op[m], woT[:, kt, m * P:(m + 1) * P],
                                d_sb[:, ktl, :],
                                start=False, stop=(pi == 2 and ktl == 3))
                # epilogue + store (alternate engines)
                ysb = ybuf.tile([P, 4, NT], F32, tag="y")
                for m in range(4):
                    if m % 2 == 0:
                        nc.vector.tensor_scalar(
                            ysb[:, m, :], op[m], bo[:, m:m + 1], 0.0,
                            op0=mybir.AluOpType.add, op1=mybir.AluOpType.max)
                    else:
                        nc.scalar.activation(ysb[:, m, :], op[m], RELU,
                                             bias=bo[:, m:m + 1], scale=1.0)
                    dma_eng = nc.sync if m % 2 == 0 else nc.scalar
                    dma_eng.dma_start(
                        y_d[m * P:(m + 1) * P, b * NT:(b + 1) * NT],
                        ysb[:, m, :])
    return nc


# ---------------------------------------------------------------------------
# Host side
# ---------------------------------------------------------------------------

_NC_CACHE = {}


def _get_nc():
    if "nc" not in _NC_CACHE:
        _NC_CACHE["nc"] = _build_bass()
    return _NC_CACHE["nc"]


def _host_prep(inputs):
    """Fold BN scales into weights, transpose into partition-major SBUF
    layouts, cast bf16."""
    bf16 = ml_dtypes.bfloat16
    f32 = np.float32

    def A(name):
        return np.asarray(inputs[name], f32)

    # lhsT for conv1: [K=C, M=C4] per k -> [P, 3, 4, C4]
    w1T = np.stack([(A(f"s1_{k}")[:, None] * A(f"w1_{k}")).T for k in (1, 3, 5)])
    w1sb = w1T.reshape(3, 4, P, C4).transpose(2, 0, 1, 3)
    areas = {1: 3600.0, 3: 400.0, 5: 144.0}
    w2T = np.stack([((A(f"s2_{k}")[:, None] * A(f"w2_{k}")) / areas[k]).T
                    for k in (1, 3, 5)])
    w2sb = w2T.reshape(3, 4, P, C4).transpose(2, 0, 1, 3)
    # lhsT for d conv: [K=C4, M=C] per k -> [P, 3, C]
    wfT = np.stack([(A(f"sf_{k}")[:, None] * A(f"wf_{k}")).T for k in (1, 3, 5)])
    wfsb = wfT.transpose(1, 0, 2)
    # lhsT for out conv: [K=4C, M=C] -> [P, 16, C]
    woT = (A("s_out")[:, None] * A("w_out")).T
    wosb = woT.reshape(16, P, C).transpose(1, 0, 2)

    b1sb = np.stack([A(f"b1_{k}") for k in (1, 3, 5)]).T
    b2sb = np.stack([A(f"b2_{k}") for k in (1, 3, 5)]).T
    bfsb = np.stack([A(f"bf_{k}").reshape(4, P) for k in (1, 3, 5)]).transpose(2, 0, 1)
    bosb = A("b_out").reshape(4, P).T
    return {
        "w1sb": np.ascontiguousarray(w1sb).astype(bf16),
        "w2sb": np.ascontiguousarray(w2sb).astype(bf16),
        "wfsb": np.ascontiguousarray(wfsb).astype(bf16),
        "wosb": np.ascontiguousarray(wosb).astype(bf16),
        "b1sb": np.ascontiguousarray(b1sb),
        "b2sb": np.ascontiguousarray(b2sb),
        "bfsb": np.ascontiguousarray(bfsb),
        "bosb": np.ascontiguousarray(bosb),
        "ident": np.eye(P, dtype=f32),
    }


def _host_x(x):
    """[512, 3600] fp32 -> chunk-major [NCHUNK, P, 4, CHUNK] bf16."""
    xb = x.astype(ml_dtypes.bfloat16)
    # row = kt*128 + p ; col = cb*CHUNK + w
    return np.ascontiguousarray(
        xb.reshape(4, P, NCHUNK, CHUNK).transpose(2, 1, 0, 3))


def _run(inputs, **kwargs):
    from concourse.bass_utils import run_bass_kernel_spmd

    common = _host_prep(inputs)
    x = np.asarray(inputs["x"], np.float32).reshape(N_CORES, C, HW)
    in_maps = [{**common, "x": _host_x(x[n])} for n in range(N_CORES)]
    return run_bass_kernel_spmd(_get_nc(), in_maps,
                                core_ids=list(range(N_CORES)), **kwargs)


def kernel(**inputs):
    res = _run(inputs)
    return np.stack([r["y"].reshape(C, H, W) for r in res.results]).astype(np.float32)
